# revision 14
# baseline (speedup 1.0000x reference)
"""Bahdanau-style attention kernel for Trainium2, 8 NeuronCores.

Reference computation (per batch b):
    score  = tanh(c @ W1 + W1_b + (h @ W2 + W2_b)[None, :])   # [T, U]
    logits = score @ V_w (+ V_b, cancels in softmax)          # [T, 1]
    attn   = softmax(logits over T)
    out    = sum_t attn[t] * c[t, :]                          # [D]

Sharding: pure data-parallel over batch B=64 across 8 cores (8 batches/core).
No collectives needed; host concatenates per-core outputs.

Per-core dataflow (B_loc=8, T=2048, D=U=512):
  - stream c in 512-row super-tiles [128p, (4s d)] fp32, cast to bf16 on DVE
  - build cT tiles via 16x SBUF->SBUF DMA-transpose (bf16, HWDGE)
  - main matmul on TensorE: psum[t128, u512] += cT_chunk.T @ W1_chunk (bf16)
  - + h-projection broadcast via K=1 matmul (ones x hb_row)
  - tanh on ScalarE -> score bf16
  - V-dot on VectorE (fused tensor_tensor_reduce) -> logit [t,1] fp32
  - exp on ScalarE (no max subtraction: |logit| <= ||V||_1 ~ 18, fp32-safe)
  - pass-2 on TensorE: psum_o[1, 512] += w_col.T @ c_tile  (softmax numerator)
    and psum_sw[1,1] += w_col.T @ ones_col (softmax denominator)
  - finalize per batch: out_row = psum_o * (1/psum_sw) on DVE, DMA out.
"""

import numpy as np

import concourse.bass as bass
import concourse.tile as tile
from concourse import bacc, mybir
from concourse import bass_utils

B, T, D, U = 64, 2048, 512, 512
NCORES = 8
BL = B // NCORES  # 8 batches per core
KD = D // 128     # 4 contraction chunks
NST = T // 512    # 4 super-tiles per batch
F32 = mybir.dt.float32
BF16 = mybir.dt.bfloat16
AF = mybir.ActivationFunctionType
ALU = mybir.AluOpType


def build_nc(n_batch=BL, n_st=NST, skip_hbflat_dma=False, repeat=1, stage=7):
    # stage: 1=c-load 2=+cast 3=+transposes 4=+main-mms 5=+tanh 6=+vdot 7=full
    nc = bacc.Bacc(None, target_bir_lowering=False)

    c_ext = nc.declare_dram_parameter("c", [BL, T, D], F32, isOutput=False)
    h_ext = nc.declare_dram_parameter("h", [BL, D], F32, isOutput=False)
    w1_ext = nc.declare_dram_parameter("W1_w", [D, U], F32, isOutput=False)
    b1_ext = nc.declare_dram_parameter("W1_b", [U], F32, isOutput=False)
    w2_ext = nc.declare_dram_parameter("W2_w", [D, U], F32, isOutput=False)
    b2_ext = nc.declare_dram_parameter("W2_b", [U], F32, isOutput=False)
    v_ext = nc.declare_dram_parameter("V_w", [U, 1], F32, isOutput=False)
    ones_ext = nc.declare_dram_parameter("ones", [128, 128], F32, isOutput=False)
    out_ext = nc.declare_dram_parameter("out", [BL, D], F32, isOutput=True)

    with tile.TileContext(nc) as tc:
        with (
            tc.tile_pool(name="const", bufs=1) as constp,
            tc.tile_pool(name="cin", bufs=3) as cinp,
            tc.tile_pool(name="cbf", bufs=3) as cbfp,
            tc.tile_pool(name="ct", bufs=3) as ctp,
            tc.tile_pool(name="work", bufs=4) as workp,
        ):
            # ---------------- setup (one-time) ----------------
            with tc.tile_pool(name="spsum", bufs=1, space="PSUM") as sps:
                # constants: ones (f32 -> bf16)
                ones_f = constp.tile([128, 128], F32)
                nc.gpsimd.dma_start(ones_f[:], ones_ext[:, :])
                ones_bf = constp.tile([128, 128], BF16)
                nc.scalar.activation(ones_bf[:], ones_f[:], AF.Copy)

                # W1, W2 in [p, (k u)] layout, cast to bf16
                w1_f = constp.tile([128, KD * U], F32)
                nc.gpsimd.dma_start(
                    w1_f[:].rearrange("p (k u) -> p k u", k=KD),
                    w1_ext.rearrange("(k p) u -> p k u", p=128),
                )
                w1_bf = constp.tile([128, KD * U], BF16)
                nc.vector.tensor_copy(w1_bf[:], w1_f[:])

                w2_f = constp.tile([128, KD * U], F32)
                nc.gpsimd.dma_start(
                    w2_f[:].rearrange("p (k u) -> p k u", k=KD),
                    w2_ext.rearrange("(k p) u -> p k u", p=128),
                )
                w2_bf = constp.tile([128, KD * U], BF16)
                nc.vector.tensor_copy(w2_bf[:], w2_f[:])

                # h [BL, D] -> padded [16, D] bf16 -> hT [128, (k 16)] via DMA-T
                h_f = constp.tile([16, D], F32)
                nc.vector.memset(h_f[:], 0.0)
                nc.gpsimd.dma_start(h_f[0:BL, :], h_ext[:, :])
                h_bf = constp.tile([16, D], BF16)
                nc.vector.tensor_copy(h_bf[:], h_f[:])
                hT_bf = constp.tile([128, KD * 16], BF16)
                for k in range(KD):
                    nc.sync.dma_start(
                        out=hT_bf[:, 16 * k : 16 * (k + 1)],
                        in_=h_bf[0:16, 128 * k : 128 * (k + 1)],
                        transpose=True,
                    )

                # biases: b12 = W1_b + W2_b -> bf16 [1, U]
                b1_f = constp.tile([1, U], F32)
                nc.gpsimd.dma_start(b1_f[:], b1_ext[None, :])
                b2_f = constp.tile([1, U], F32)
                nc.gpsimd.dma_start(b2_f[:], b2_ext[None, :])
                b12_f = constp.tile([1, U], F32)
                nc.vector.tensor_add(b12_f[:], b1_f[:], b2_f[:])
                b12_bf = constp.tile([1, U], BF16)
                nc.scalar.activation(b12_bf[:], b12_f[:], AF.Copy)

                # V row [1, U] f32 -> bf16 -> broadcast [128, U] bf16
                v_f = constp.tile([1, U], F32)
                nc.gpsimd.dma_start(v_f[:], v_ext.rearrange("u o -> o u"))
                v_bf = constp.tile([1, U], BF16)
                nc.scalar.activation(v_bf[:], v_f[:], AF.Copy)
                ps_vb = sps.tile([128, U], F32)
                nc.tensor.matmul(
                    ps_vb[:], lhsT=ones_bf[0:1, :], rhs=v_bf[:], start=True, stop=True
                )
                vbc_bf = constp.tile([128, U], BF16)
                nc.scalar.activation(vbc_bf[:], ps_vb[:], AF.Copy)

                # hb[b, u] = h[b] @ W2 + W1_b + W2_b  (rows 0:BL valid)
                ps_hb = sps.tile([16, U], F32)
                for k in range(KD):
                    nc.tensor.matmul(
                        ps_hb[:],
                        lhsT=hT_bf[:, 16 * k : 16 * (k + 1)],
                        rhs=w2_bf[:, U * k : U * (k + 1)],
                        start=(k == 0),
                        stop=False,
                    )
                nc.tensor.matmul(
                    ps_hb[:], lhsT=ones_bf[0:1, 0:16], rhs=b12_bf[:],
                    start=False, stop=True,
                )
                hb_rows = constp.tile([16, U], BF16)
                nc.scalar.activation(hb_rows[:], ps_hb[:], AF.Copy)
                # flatten to single-partition [1, BL*U] so per-batch slices
                # have base_partition 0 (matmul rhs requirement)
                hb_flat = constp.tile([1, BL * U], BF16)
                if skip_hbflat_dma:
                    nc.vector.memset(hb_flat[:], 0.0)
                else:
                    for b in range(BL):
                        nc.gpsimd.dma_start(
                            hb_flat[0:1, U * b : U * (b + 1)], hb_rows[b : b + 1, :]
                        )

            # ---------------- main loop ----------------
            with (
                tc.tile_pool(name="psum_s", bufs=4, space="PSUM") as psp,
                tc.tile_pool(name="psum_o", bufs=2, space="PSUM") as pop,
                tc.tile_pool(name="psum_w", bufs=2, space="PSUM") as pwp,
            ):
                for rep in range(repeat):
                  for b in range(n_batch):
                    psum_o = pop.tile([1, D], F32)
                    psum_sw = pwp.tile([1, 1], F32)
                    for st in range(n_st):
                        t0 = 512 * st
                        cf = cinp.tile([128, 4 * D], F32)
                        nc.gpsimd.dma_start(
                            cf[:].rearrange("p (s d) -> p s d", s=4),
                            c_ext[b, t0 : t0 + 512, :].rearrange(
                                "(s p) d -> p s d", p=128
                            ),
                        )
                        if stage < 2:
                            continue
                        cb = cbfp.tile([128, 4 * D], BF16)
                        nc.vector.tensor_copy(cb[:], cf[:])
                        if stage < 3:
                            continue
                        ct = ctp.tile([128, KD * 512], BF16)
                        for s in range(4):
                            for k in range(KD):
                                nc.sync.dma_start(
                                    out=ct[:, 512 * k + 128 * s : 512 * k + 128 * (s + 1)],
                                    in_=cb[:, D * s + 128 * k : D * s + 128 * (k + 1)],
                                    transpose=True,
                                )
                        if stage < 4:
                            continue
                        for s in range(4):
                            ps = psp.tile([128, U], F32)
                            for k in range(KD):
                                nc.tensor.matmul(
                                    ps[:],
                                    lhsT=ct[:, 512 * k + 128 * s : 512 * k + 128 * (s + 1)],
                                    rhs=w1_bf[:, U * k : U * (k + 1)],
                                    start=(k == 0),
                                    stop=False,
                                )
                            nc.tensor.matmul(
                                ps[:],
                                lhsT=ones_bf[0:1, :],
                                rhs=hb_flat[0:1, U * b : U * (b + 1)],
                                start=False,
                                stop=True,
                            )
                            if stage < 5:
                                continue
                            score = workp.tile([128, U], BF16, tag="score")
                            nc.scalar.activation(score[:], ps[:], AF.Tanh)
                            if stage < 6:
                                continue
                            prod = workp.tile([128, U], BF16, tag="prod")
                            nc.vector.tensor_mul(prod[:], score[:], vbc_bf[:])
                            logit = workp.tile([128, 1], F32, tag="logit")
                            nc.vector.reduce_sum(
                                logit[:], prod[:], axis=mybir.AxisListType.X
                            )
                            if stage < 7:
                                continue
                            wcol = workp.tile([128, 1], BF16, tag="wcol")
                            nc.scalar.activation(wcol[:], logit[:], AF.Exp)
                            first = st == 0 and s == 0
                            last = st == n_st - 1 and s == 3
                            nc.tensor.matmul(
                                psum_o[:],
                                lhsT=wcol[:],
                                rhs=cb[:, D * s : D * (s + 1)],
                                start=first,
                                stop=last,
                            )
                            nc.tensor.matmul(
                                psum_sw[:],
                                lhsT=wcol[:],
                                rhs=ones_bf[:, 0:1],
                                start=first,
                                stop=last,
                            )
                    if stage >= 7:
                        inv = workp.tile([1, 1], F32, tag="inv")
                        nc.vector.reciprocal(inv[:], psum_sw[0:1, 0:1])
                        orow = workp.tile([1, D], F32, tag="orow")
                        nc.vector.tensor_scalar_mul(
                            orow[:], psum_o[0:1, :], inv[0:1, 0:1]
                        )
                        nc.gpsimd.dma_start(out_ext[b : b + 1, :], orow[:])
    nc.compile()
    return nc


_NC_CACHE = None


def _get_nc():
    global _NC_CACHE
    if _NC_CACHE is None:
        _NC_CACHE = build_nc()
    return _NC_CACHE


def kernel(**inputs):
    c = np.asarray(inputs["c"], dtype=np.float32)
    h = np.asarray(inputs["h"], dtype=np.float32)
    shared = {
        "W1_w": np.ascontiguousarray(np.asarray(inputs["W1_w"], np.float32)),
        "W1_b": np.ascontiguousarray(np.asarray(inputs["W1_b"], np.float32)),
        "W2_w": np.ascontiguousarray(np.asarray(inputs["W2_w"], np.float32)),
        "W2_b": np.ascontiguousarray(np.asarray(inputs["W2_b"], np.float32)),
        "V_w": np.ascontiguousarray(np.asarray(inputs["V_w"], np.float32)),
        "ones": np.ones((128, 128), np.float32),
    }
    in_maps = []
    for i in range(NCORES):
        m = dict(shared)
        m["c"] = np.ascontiguousarray(c[i * BL : (i + 1) * BL])
        m["h"] = np.ascontiguousarray(h[i * BL : (i + 1) * BL])
        in_maps.append(m)

    nc = _get_nc()
    res = bass_utils.run_bass_kernel_spmd(nc, in_maps, core_ids=list(range(NCORES)))
    out = np.concatenate([np.asarray(r["out"]) for r in res.results], axis=0)
    return out.astype(np.float32)


if __name__ == "__main__":
    rng = np.random.default_rng(0)
    ins = {
        "c": rng.standard_normal((B, T, D), dtype=np.float32),
        "h": rng.standard_normal((B, D), dtype=np.float32),
        "W1_w": rng.standard_normal((D, U), dtype=np.float32) / np.sqrt(D),
        "W1_b": np.zeros((U,), np.float32),
        "W2_w": rng.standard_normal((D, U), dtype=np.float32) / np.sqrt(D),
        "W2_b": np.zeros((U,), np.float32),
        "V_w": rng.standard_normal((U, 1), dtype=np.float32) / np.sqrt(U),
        "V_b": np.zeros((1,), np.float32),
    }
    out = kernel(**ins)
    print("out", out.shape, out.dtype, np.abs(out).mean())


# revision 15
# speedup vs baseline: 2.0252x; 2.0252x over previous
"""Bahdanau-style attention kernel for Trainium2, 8 NeuronCores.

Reference computation (per batch b):
    score  = tanh(c @ W1 + W1_b + (h @ W2 + W2_b)[None, :])   # [T, U]
    logits = score @ V_w (+ V_b, cancels in softmax)          # [T, 1]
    attn   = softmax(logits over T)
    out    = sum_t attn[t] * c[t, :]                          # [D]

Sharding: pure data-parallel over batch B=64 across 8 cores (8 batches/core).
No collectives; host concatenates per-core outputs.

Per-core dataflow (B_loc=8, T=2048, D=U=512), per batch:
  - stream c in 512-row super-tiles [128p, (4s d)] fp32, cast to bf16 on DVE
  - store bf16 c back to a DRAM scratch tile (natural [T, D] layout)
  - 4 big DMA-transpose loads  cT[d128, T] <- scratch[:, d-chunk]  (HWDGE xbar)
  - main matmul on TensorE: psum[t128, u512] += cT_chunk.T @ W1_chunk (bf16)
    + h-projection broadcast via K=1 matmul (ones x hb_row)
  - tanh on ScalarE -> score bf16
  - V-dot on VectorE (mult + reduce) -> logit [t,1] fp32
  - exp on ScalarE (no max subtraction: |logit| <= ||V||_1, fp32-safe)
  - pass-2 on TensorE: psum_o[1, 512] += w_col.T @ c_bf16_tile (numerator)
    and psum_sw[1,1] += w_col.T @ ones_col (denominator)
  - finalize per batch: out_row = psum_o * (1/psum_sw) on DVE, DMA out.
"""

import numpy as np

import concourse.bass as bass
import concourse.tile as tile
from concourse import bacc, mybir
from concourse import bass_utils

B, T, D, U = 64, 2048, 512, 512
NCORES = 8
BL = B // NCORES  # 8 batches per core
KD = D // 128     # 4 contraction chunks
NST = T // 512    # 4 super-tiles per batch
F32 = mybir.dt.float32
BF16 = mybir.dt.bfloat16
AF = mybir.ActivationFunctionType
ALU = mybir.AluOpType


def build_nc(n_batch=BL, n_st=NST, repeat=1, stage=7):
    # stage: 1=c-load 2=+cast 3=+store 4=+transpose 5=+main-mms 6=+tanh+vdot 7=full
    assert n_st == NST, "DRAM-transpose scheme needs full T per batch"
    nc = bacc.Bacc(None, target_bir_lowering=False)

    c_ext = nc.declare_dram_parameter("c", [BL, T, D], F32, isOutput=False)
    h_ext = nc.declare_dram_parameter("h", [BL, D], F32, isOutput=False)
    w1_ext = nc.declare_dram_parameter("W1_w", [D, U], F32, isOutput=False)
    b1_ext = nc.declare_dram_parameter("W1_b", [U], F32, isOutput=False)
    w2_ext = nc.declare_dram_parameter("W2_w", [D, U], F32, isOutput=False)
    b2_ext = nc.declare_dram_parameter("W2_b", [U], F32, isOutput=False)
    v_ext = nc.declare_dram_parameter("V_w", [U, 1], F32, isOutput=False)
    ones_ext = nc.declare_dram_parameter("ones", [128, 128], F32, isOutput=False)
    out_ext = nc.declare_dram_parameter("out", [BL, D], F32, isOutput=True)

    with tile.TileContext(nc) as tc:
        with (
            tc.tile_pool(name="const", bufs=1) as constp,
            tc.tile_pool(name="cin", bufs=3) as cinp,
            tc.tile_pool(name="cbf", bufs=8) as cbfp,
            tc.tile_pool(name="ct", bufs=2) as ctp,
            tc.tile_pool(name="work", bufs=4) as workp,
            tc.tile_pool(name="dram", bufs=2, space="DRAM") as dramp,
        ):
            # ---------------- setup (one-time) ----------------
            with tc.tile_pool(name="spsum", bufs=1, space="PSUM") as sps:
                ones_f = constp.tile([128, 128], F32)
                nc.gpsimd.dma_start(ones_f[:], ones_ext[:, :])
                ones_bf = constp.tile([128, 128], BF16)
                nc.scalar.activation(ones_bf[:], ones_f[:], AF.Copy)

                w1_f = constp.tile([128, KD * U], F32)
                nc.gpsimd.dma_start(
                    w1_f[:].rearrange("p (k u) -> p k u", k=KD),
                    w1_ext.rearrange("(k p) u -> p k u", p=128),
                )
                w1_bf = constp.tile([128, KD * U], BF16)
                nc.vector.tensor_copy(w1_bf[:], w1_f[:])

                w2_f = constp.tile([128, KD * U], F32)
                nc.gpsimd.dma_start(
                    w2_f[:].rearrange("p (k u) -> p k u", k=KD),
                    w2_ext.rearrange("(k p) u -> p k u", p=128),
                )
                w2_bf = constp.tile([128, KD * U], BF16)
                nc.vector.tensor_copy(w2_bf[:], w2_f[:])

                h_f = constp.tile([16, D], F32)
                nc.vector.memset(h_f[:], 0.0)
                nc.gpsimd.dma_start(h_f[0:BL, :], h_ext[:, :])
                h_bf = constp.tile([16, D], BF16)
                nc.vector.tensor_copy(h_bf[:], h_f[:])
                hT_bf = constp.tile([128, KD * 16], BF16)
                for k in range(KD):
                    nc.sync.dma_start(
                        out=hT_bf[:, 16 * k : 16 * (k + 1)],
                        in_=h_bf[0:16, 128 * k : 128 * (k + 1)],
                        transpose=True,
                    )

                b1_f = constp.tile([1, U], F32)
                nc.gpsimd.dma_start(b1_f[:], b1_ext[None, :])
                b2_f = constp.tile([1, U], F32)
                nc.gpsimd.dma_start(b2_f[:], b2_ext[None, :])
                b12_f = constp.tile([1, U], F32)
                nc.vector.tensor_add(b12_f[:], b1_f[:], b2_f[:])
                b12_bf = constp.tile([1, U], BF16)
                nc.scalar.activation(b12_bf[:], b12_f[:], AF.Copy)

                v_f = constp.tile([1, U], F32)
                nc.gpsimd.dma_start(v_f[:], v_ext.rearrange("u o -> o u"))
                v_bf = constp.tile([1, U], BF16)
                nc.scalar.activation(v_bf[:], v_f[:], AF.Copy)
                ps_vb = sps.tile([128, U], F32)
                nc.tensor.matmul(
                    ps_vb[:], lhsT=ones_bf[0:1, :], rhs=v_bf[:], start=True, stop=True
                )
                vbc_bf = constp.tile([128, U], BF16)
                nc.scalar.activation(vbc_bf[:], ps_vb[:], AF.Copy)

                ps_hb = sps.tile([16, U], F32)
                for k in range(KD):
                    nc.tensor.matmul(
                        ps_hb[:],
                        lhsT=hT_bf[:, 16 * k : 16 * (k + 1)],
                        rhs=w2_bf[:, U * k : U * (k + 1)],
                        start=(k == 0),
                        stop=False,
                    )
                nc.tensor.matmul(
                    ps_hb[:], lhsT=ones_bf[0:1, 0:16], rhs=b12_bf[:],
                    start=False, stop=True,
                )
                hb_rows = constp.tile([16, U], BF16)
                nc.scalar.activation(hb_rows[:], ps_hb[:], AF.Copy)
                hb_flat = constp.tile([1, BL * U], BF16)
                for b in range(BL):
                    nc.gpsimd.dma_start(
                        hb_flat[0:1, U * b : U * (b + 1)], hb_rows[b : b + 1, :]
                    )

            # ---------------- main loop ----------------
            with (
                tc.tile_pool(name="psum_s", bufs=4, space="PSUM") as psp,
                tc.tile_pool(name="psum_o", bufs=2, space="PSUM") as pop,
                tc.tile_pool(name="psum_w", bufs=2, space="PSUM") as pwp,
            ):
                for rep in range(repeat):
                  for b in range(n_batch):
                    cbts = []
                    cbf_d = dramp.tile([T, D], BF16)
                    for st in range(NST):
                        t0 = 512 * st
                        cf = cinp.tile([128, 4 * D], F32)
                        nc.gpsimd.dma_start(
                            cf[:].rearrange("p (s d) -> p s d", s=4),
                            c_ext[b, t0 : t0 + 512, :].rearrange(
                                "(s p) d -> p s d", p=128
                            ),
                        )
                        if stage < 2:
                            continue
                        cbt = cbfp.tile([128, 4 * D], BF16)
                        nc.vector.tensor_copy(cbt[:], cf[:])
                        cbts.append(cbt)
                        if stage < 3:
                            continue
                        nc.gpsimd.dma_start(
                            cbf_d[t0 : t0 + 512, :].rearrange(
                                "(s p) d -> p s d", p=128
                            ),
                            cbt[:].rearrange("p (s d) -> p s d", s=4),
                        )
                    if stage < 4:
                        continue
                    cts = ctp.tile([128, KD * T], BF16)
                    for k in range(KD):
                        nc.sync.dma_start(
                            out=cts[:, T * k : T * (k + 1)],
                            in_=cbf_d[:, 128 * k : 128 * (k + 1)],
                            transpose=True,
                        )
                    if stage < 5:
                        continue
                    psum_o = pop.tile([1, D], F32)
                    psum_sw = pwp.tile([1, 1], F32)
                    for st in range(NST):
                        for s in range(4):
                            tb = 128 * (4 * st + s)  # t-block offset within T
                            ps = psp.tile([128, U], F32)
                            for k in range(KD):
                                nc.tensor.matmul(
                                    ps[:],
                                    lhsT=cts[:, T * k + tb : T * k + tb + 128],
                                    rhs=w1_bf[:, U * k : U * (k + 1)],
                                    start=(k == 0),
                                    stop=False,
                                )
                            nc.tensor.matmul(
                                ps[:],
                                lhsT=ones_bf[0:1, :],
                                rhs=hb_flat[0:1, U * b : U * (b + 1)],
                                start=False,
                                stop=True,
                            )
                            if stage < 6:
                                continue
                            score = workp.tile([128, U], BF16, tag="score")
                            nc.scalar.activation(score[:], ps[:], AF.Tanh)
                            prod = workp.tile([128, U], BF16, tag="prod")
                            nc.vector.tensor_mul(prod[:], score[:], vbc_bf[:])
                            logit = workp.tile([128, 1], F32, tag="logit")
                            nc.vector.reduce_sum(
                                logit[:], prod[:], axis=mybir.AxisListType.X
                            )
                            if stage < 7:
                                continue
                            wcol = workp.tile([128, 1], BF16, tag="wcol")
                            nc.scalar.activation(wcol[:], logit[:], AF.Exp)
                            first = st == 0 and s == 0
                            last = st == NST - 1 and s == 3
                            nc.tensor.matmul(
                                psum_o[:],
                                lhsT=wcol[:],
                                rhs=cbts[st][:, D * s : D * (s + 1)],
                                start=first,
                                stop=last,
                            )
                            nc.tensor.matmul(
                                psum_sw[:],
                                lhsT=wcol[:],
                                rhs=ones_bf[:, 0:1],
                                start=first,
                                stop=last,
                            )
                    if stage >= 7:
                        inv = workp.tile([1, 1], F32, tag="inv")
                        nc.vector.reciprocal(inv[:], psum_sw[0:1, 0:1])
                        orow = workp.tile([1, D], F32, tag="orow")
                        nc.vector.tensor_scalar_mul(
                            orow[:], psum_o[0:1, :], inv[0:1, 0:1]
                        )
                        nc.gpsimd.dma_start(out_ext[b : b + 1, :], orow[:])
    nc.compile()
    return nc


_NC_CACHE = None


def _get_nc():
    global _NC_CACHE
    if _NC_CACHE is None:
        _NC_CACHE = build_nc()
    return _NC_CACHE


def kernel(**inputs):
    c = np.asarray(inputs["c"], dtype=np.float32)
    h = np.asarray(inputs["h"], dtype=np.float32)
    shared = {
        "W1_w": np.ascontiguousarray(np.asarray(inputs["W1_w"], np.float32)),
        "W1_b": np.ascontiguousarray(np.asarray(inputs["W1_b"], np.float32)),
        "W2_w": np.ascontiguousarray(np.asarray(inputs["W2_w"], np.float32)),
        "W2_b": np.ascontiguousarray(np.asarray(inputs["W2_b"], np.float32)),
        "V_w": np.ascontiguousarray(np.asarray(inputs["V_w"], np.float32)),
        "ones": np.ones((128, 128), np.float32),
    }
    in_maps = []
    for i in range(NCORES):
        m = dict(shared)
        m["c"] = np.ascontiguousarray(c[i * BL : (i + 1) * BL])
        m["h"] = np.ascontiguousarray(h[i * BL : (i + 1) * BL])
        in_maps.append(m)

    nc = _get_nc()
    res = bass_utils.run_bass_kernel_spmd(nc, in_maps, core_ids=list(range(NCORES)))
    out = np.concatenate([np.asarray(r["out"]) for r in res.results], axis=0)
    return out.astype(np.float32)


if __name__ == "__main__":
    rng = np.random.default_rng(0)
    ins = {
        "c": rng.standard_normal((B, T, D), dtype=np.float32),
        "h": rng.standard_normal((B, D), dtype=np.float32),
        "W1_w": rng.standard_normal((D, U), dtype=np.float32) / np.sqrt(D),
        "W1_b": np.zeros((U,), np.float32),
        "W2_w": rng.standard_normal((D, U), dtype=np.float32) / np.sqrt(D),
        "W2_b": np.zeros((U,), np.float32),
        "V_w": rng.standard_normal((U, 1), dtype=np.float32) / np.sqrt(U),
        "V_b": np.zeros((1,), np.float32),
    }
    out = kernel(**ins)
    print("out", out.shape, out.dtype, np.abs(out).mean())
